# revision 56
# baseline (speedup 1.0000x reference)
"""AttentionBlock (GroupNorm + single-head 4096x4096 attention + proj + residual)
on 8 Trainium2 NeuronCores.

Sharding: core c = 2*b + h handles image b (of 4), query-half h (of 2).
Each core:
  - receives x pre-transposed to channel-major bf16 (host does the transpose),
  - computes GroupNorm statistics via bn_stats while x streams in,
  - computes kT [512,4096] and v [4096,512] for the full image (k/v duplicated
    across the half-pair, ~10% extra FLOPs, no collectives),
  - computes qT for its 2048 query rows,
  - attention over its 2048 queries, projection + bias + residual for its rows.

Precision: fp8e4m3 (TRN 240-max) with MatmulPerfMode.DoubleRow for ALL GEMMs
(hn/q/k/v/scores/PV/rowsum/proj), fp32 PSUM accumulation, fp32 GroupNorm
statistics, fp32 softmax row-sums / normalization, fp32 residual.

fp8 scale bookkeeping:
  - wq/wk/wv/wp host-scaled x16 (keeps N(0, 1/sqrt(C)) weights out of the fp8
    subnormal range); biases bq/bk/bv host-scaled x16 to match.
  - qT/kT hold 16q/16k; scores PSUM = 256*q.k; exp scale folds the 1/256.
  - exp has bias -2.0 (max score 6.81 -> et max e^4.81=123 < 240 fp8 max);
    the e^-2 factor cancels between numerator and row-sum.
  - vS holds 16v; po PSUM = 16*sum(et*v); ot eviction scales by 1/128.
  - proj PSUM py = (po/128) @ (16 wp) = 2*sum(et*v)@wp; rowsum matmul uses a
    2.0-valued ones vector so rt = 1/pr = 1/(2*sum(et)) normalizes exactly.
"""

import sys

sys.path.insert(0, "/opt/trn_rl_repo")

import numpy as np  # noqa: E402

import bass_rust  # noqa: E402
import concourse.bass as bass  # noqa: E402
import concourse.mybir as mybir  # noqa: E402
import concourse.tile as tile  # noqa: E402
from concourse.vector_clock import ScopedClock  # noqa: E402
from concourse.bass_utils import run_bass_kernel_spmd  # noqa: E402

F32 = mybir.dt.float32
BF16 = mybir.dt.bfloat16
F8 = mybir.dt.float8e4
AF = mybir.ActivationFunctionType
OP = mybir.AluOpType
DR = mybir.MatmulPerfMode.DoubleRow

B, H, W, C = 4, 64, 64, 512
HW = H * W            # 4096 positions per image
HALF = HW // 2        # 2048 query rows per core
GROUPS = 32
GSIZE = C // GROUPS   # 16 channels per group
EPS = 1e-5
N_CORES = 8
CT = C // 128         # 4 channel partition-tiles
JT = HW // 128        # 32 position partition-tiles
JC = HW // 512        # 8 position chunks (kT/v build)
QC = HALF // 512      # 4 query chunks (qT build)
IB = HALF // 512      # 4 query i-blocks (attention)
WSC = 16.0            # host-side weight/bias scale (fp8 subnormal avoidance)
SM8 = 1.0 / (WSC * WSC * float(np.sqrt(C)))   # exp scale on (16q).(16k) psum
EXPB = -2.0           # exp bias: keeps et = e^(s-2) <= e^4.9 < 240 (fp8 max)
OTS = 1.0 / 128.0     # po -> ot eviction scale (fp8 range)
ONESV = 2.0           # rowsum weights: pr = 2*sum(et) so rt=1/pr normalizes
                      # py = (po/128)@(16wp) = 2*sum(et*v)@wp exactly


# --- workaround: walrus in this container rejects instructions carrying more
# than one sync-wait command.  Move extra waits onto same-engine NOPs placed
# immediately before the instruction (engine program order makes this exact).
def _split_multi_waits(nc, max_waits=1):
    n = 0
    for f in nc.m.functions:
        for bb in f.blocks:
            newlist = []
            for inst in bb.instructions:
                si = inst.sync_info
                waits = list(si.on_wait) if si is not None else []
                if len(waits) > max_waits:
                    n += 1
                    for k, wt in enumerate(waits[:-max_waits]):
                        nop = bass_rust.InstNoOp(
                            name=f"{inst.name}-sw{k}", engine=inst.engine)
                        nop.sync_info = mybir.SyncInfo(on_wait=[wt], on_update=[])
                        newlist.append(nop)
                    inst.sync_info = mybir.SyncInfo(
                        on_wait=waits[-max_waits:], on_update=list(si.on_update))
                newlist.append(inst)
            bb.instructions[:] = newlist
    return n


def _split_drain_and_barrier(self, tick_clock, wait_clock):
    # same as TileContext._drain_and_barrier but with the tail drain's waits
    # split onto single-wait NOPs (same walrus limitation as above).
    drain_inst = self.nc.sync.drain()
    wait_clock.add_sem_waits(
        drain_inst.ins, ScopedClock({None: tick_clock.global_clock}))
    mi = drain_inst.ins
    waits = list(mi.sync_info.on_wait) if mi.sync_info is not None else []
    if len(waits) > 1:
        mi.sync_info.on_wait = []
        for wt in waits:
            wi = self.nc.sync.nop(nofuse=True, hint="tail_drain_wait")
            wi.ins.sync_info = mybir.SyncInfo(on_wait=[wt], on_update=[])
    self.nc.all_engine_barrier()
    assert self.sems is not None
    popped = self.nc._tile_sem_poison_stack.pop()
    assert popped is self._sem_poison
    self.nc.clear_and_free_semaphores(list(self.sems.allocated().values()))
    self.nc.all_engine_barrier()


tile.TileContext._drain_and_barrier = _split_drain_and_barrier


def build_program(split_waits=True):
    nc = bass.Bass()

    # xT rows are permuted per-core so the query half is always positions
    # [0, HALF), host-transposed to channel-major and slab-tiled
    # [2, CT, 128, 2048] fp8 so each (half, ct) slab DMA is one contiguous
    # 256KB read -- the x stream gates the GroupNorm stats chain, so halving
    # its bytes (vs bf16) shortens the serial startup.  fp8 x only perturbs
    # hn by ~3% (already the hnT quantization level) and GN variance by 0.1%.
    xTd = nc.dram_tensor("xT", [2, CT, 128, 2048], F8, kind="ExternalInput")
    xq = nc.dram_tensor("xq", [HALF, C], F32, kind="ExternalInput")
    wq = nc.dram_tensor("wq", [C, C], BF16, kind="ExternalInput")
    wk = nc.dram_tensor("wk", [C, C], BF16, kind="ExternalInput")
    wv = nc.dram_tensor("wv", [C, C], BF16, kind="ExternalInput")
    wp = nc.dram_tensor("wp", [C, C], BF16, kind="ExternalInput")
    # packed per-channel constants [128, CT, 4] = (16*bq, 16*bk, gamma, beta)
    cvecd = nc.dram_tensor("cvec", [128, CT, 4], F32, kind="ExternalInput")
    # bp here is host-computed bp + bv @ wp (bv folded through the attention)
    bpd = nc.dram_tensor("bp", [C], F32, kind="ExternalInput")
    gseld = nc.dram_tensor("gsel", [GROUPS, C], F32, kind="ExternalInput")
    # gsel2[p, ct, g] = 1/GSIZE where channel ct*128+p belongs to group g
    gsel2d = nc.dram_tensor("gsel2", [128, CT, GROUPS], F32, kind="ExternalInput")
    yd = nc.dram_tensor("y", [HALF, C], F32, kind="ExternalOutput")

    xqt = xq[:, :].rearrange("(t p) c -> t p c", p=128)   # [16,128,512]
    yt = yd[:, :].rearrange("(t p) c -> t p c", p=128)    # [16,128,512]

    with tile.TileContext(nc) as tc:
        # ---------------- persistent storage + constants ----------------
        store = tc.alloc_tile_pool(name="store", bufs=1)
        kT = store.tile([128, CT, HW], F8)       # kT[c%128, c//128, j] = 16k
        vS = store.tile([128, JT, C], F8)        # v[j%128, j//128, c] = 16v
        qT = store.tile([128, CT, HALF], F8)     # qT[c%128, c//128, i] = 16q
        # x^T in fp8, one tile per (channel-tile, image-half)
        xTs = [[store.tile([128, HALF], F8, tag=f"xT{ct}_{hf}",
                           name=f"xT{ct}_{hf}") for hf in range(2)]
               for ct in range(CT)]
        wpr = store.tile([128, CT, C], F8)       # 16*wp, [cin%128, cin//128, cout]
        cst = tc.alloc_tile_pool(name="cst", bufs=1)
        gsel = cst.tile([GROUPS, C], F32)
        nc.scalar.dma_start(out=gsel, in_=gseld[:, :])
        gsel2 = cst.tile([128, CT, GROUPS], F32)
        nc.scalar.dma_start(out=gsel2, in_=gsel2d[:, :, :])
        # [128, 2, 16] so the DoubleRow weight AP's pair-dim step is 16 bytes
        # (walrus s3_lw_dual_fp8_restrictions requires step % 16 == 0)
        ones2 = cst.tile([128, 2, 16], F8)
        nc.vector.memset(ones2, ONESV)
        expb = cst.tile([128, 1], F32)
        nc.vector.memset(expb, EXPB)
        # DRAM scratch to re-layout softmax row-sums [1,512] -> [128,4]
        sumscr = nc.dram_tensor("sumscr", [IB, 512], F32)
        cv = cst.tile([128, CT, 4], F32)   # (16bq, 16bk, gamma, beta)
        nc.scalar.dma_start(out=cv, in_=cvecd[:, :, :])
        bp_bc = cst.tile([128, C], F32)
        nc.scalar.dma_start(out=bp_bc, in_=bpd[:].partition_broadcast(128))
        s_sb = cst.tile([128, CT], F32)   # GN scale per channel
        t_sb = cst.tile([128, CT], F32)   # GN shift per channel

        # 8 x-slab DMAs (512KB contiguous each) split over the sync/gpsimd
        # queues, issued FIRST (DMA issue costs ~800ns engine time each and
        # the slabs gate the GroupNorm stats chain).  ct-major order so the
        # per-ct stats aggregation below pipelines behind the DMAs.
        for ct in range(CT):
            for hf in range(2):
                xeng = nc.sync if ct % 2 == 0 else nc.gpsimd
                xeng.dma_start(out=xTs[ct][hf][:, :], in_=xTd[hf, ct, :, :])

        # weight cast bf16 -> fp8 (x16 applied on host); DMA on gpsimd queue
        # behind the x slabs; cast on the (fast) scalar engine -- Pool runs
        # fp8 casts ~5x below spec.
        wstage = tc.alloc_tile_pool(name="wstage", bufs=2)
        wrnd = tc.alloc_tile_pool(name="wrnd", bufs=1)
        wqr = wrnd.tile([128, CT, C], F8)
        wkr = wrnd.tile([128, CT, C], F8)
        wvr = wrnd.tile([128, CT, C], F8)
        for wd, wr in ((wq, wqr), (wk, wkr), (wv, wvr), (wp, wpr)):
            stg = wstage.tile([128, CT, C], BF16, tag="wstage")
            nc.gpsimd.dma_start(
                out=stg, in_=wd[:, :].rearrange("(t p) c -> p t c", p=128))
            nc.scalar.activation(wr[:, :, :], stg[:, :, :], AF.Copy)

        # ------- phase A: GroupNorm stats as the slabs land (no PE work) ----
        # ct-major: each ct's stats aggregate + feed the group matmul while
        # the next ct's slabs are still streaming in, shortening the serial
        # chain after the last bn_stats.
        with tc.tile_pool(name="pa_small", bufs=1) as pas, \
             tc.tile_pool(name="pa_ps", bufs=2, space="PSUM") as pa_ps:
            stats_sb = pas.tile([128, CT, JC, 6], F32)
            warmgate = pas.tile([128, 1], F8)
            warmgate2 = pas.tile([128, 1], F8)
            epst = pas.tile([GROUPS, 1], F32)
            nc.vector.memset(epst, EPS)
            g2 = pa_ps.tile([GROUPS, 2], F32, tag="gagg")
            mv_all = pas.tile([128, CT, 2], F32)
            sp_all = pas.tile([128, CT, 2], F32)
            for ct in range(CT):
                for jc in range(JC):
                    hf, sc = jc // 4, (jc % 4) * 512
                    nc.vector.bn_stats(
                        out=stats_sb[:, ct, jc, :],
                        in_=xTs[ct][hf][:, sc:sc + 512])
                nc.vector.bn_aggr(out=mv_all[:, ct, :], in_=stats_sb[:, ct, :, :])
                nc.vector.tensor_mul(sp_all[:, ct, 0:1], mv_all[:, ct, 0:1],
                                     mv_all[:, ct, 0:1])
                nc.vector.tensor_add(sp_all[:, ct, 1:2], sp_all[:, ct, 0:1],
                                     mv_all[:, ct, 1:2])
                nc.vector.tensor_copy(sp_all[:, ct, 0:1], mv_all[:, ct, 0:1])
                nc.tensor.matmul(g2[:, :], gsel2[:, ct, :], sp_all[:, ct, :],
                                 start=(ct == 0), stop=(ct == CT - 1))
                if ct == 2:
                    # marker late in the stats chain; the PE warm-up matmuls
                    # below are gated on it so they run during the stats tail
                    # + GN scalar chain, flipping the HAM clock gate to 2.4GHz
                    # before phase B's real matmuls arrive.
                    nc.vector.tensor_copy(warmgate[:, :], stats_sb[:, 0, 0, 1:2])
            # second marker after the whole stats chain: the follow-up warm
            # block keeps the PE busy through the ~10us group-chain window so
            # HAM does not re-throttle right before phase B's GEMMs.
            nc.vector.tensor_copy(warmgate2[:, :], stats_sb[:, 3, 0, 1:2])
            with tc.tile_pool(name="pa_warm", bufs=1, space="PSUM") as pwm:
                pw = pwm.tile([1, 512], F32)
                for r in range(24):
                    nc.tensor.matmul(
                        pw[:, :], warmgate[:, :], xTs[0][0][:, 0:512],
                        start=True, stop=True)
                for r in range(30):
                    nc.tensor.matmul(
                        pw[:, :], warmgate2[:, :], xTs[0][0][:, 0:512],
                        start=True, stop=True)

            if True:
                # group mean/var -> (mean, rstd); keep the serial chain on the
                # vector engine (one scalar hop for sqrt) -- cross-engine hops
                # cost ~0.5-1us each in queue + semaphore latency
                mv2 = pas.tile([GROUPS, 2], F32)
                nc.vector.tensor_copy(mv2[:, :], g2[:, :])   # (mean, E[x^2])
                var = pas.tile([GROUPS, 1], F32)
                nc.vector.tensor_mul(var[:, :], mv2[:, 0:1], mv2[:, 0:1])
                nc.vector.tensor_sub(var[:, :], mv2[:, 1:2], var[:, :])
                sd = pas.tile([GROUPS, 1], F32)
                nc.scalar.activation(sd[:, :], var[:, :], AF.Sqrt, bias=epst[:, :])
                nc.vector.reciprocal(mv2[:, 1:2], sd[:, :])
                # broadcast group (mean, rstd) to channels into ONE psum tile,
                # one eviction, then s/t (batched on vector)
                bc_all = pas.tile([128, CT, 2], F32)
                pbc = pa_ps.tile([128, CT, 2], F32, tag="bcast")
                for ct in range(CT):
                    nc.tensor.matmul(pbc[:, ct, :], gsel[:, ct * 128:(ct + 1) * 128],
                                     mv2[:, :], start=True, stop=True)
                nc.vector.tensor_copy(bc_all[:, :, :], pbc[:, :, :])
                nc.vector.tensor_mul(s_sb[:, :], cv[:, :, 2], bc_all[:, :, 1])
                tmp = pas.tile([128, CT], F32)
                nc.vector.tensor_mul(tmp[:, :], bc_all[:, :, 0], s_sb[:, :])
                nc.vector.tensor_sub(t_sb[:, :], cv[:, :, 3], tmp[:, :])

        # ---------------- phase B: normalize + K,V (and Q) GEMMs ------------
        def qkv_chunk(pb, pb_ps, jc):
            hnT = pb.tile([128, CT, 512], F8, tag="hnT")
            for ct in range(CT):
                # hnT = s * xT + t  (per-channel; channels on partitions)
                nc.vector.tensor_scalar(
                    hnT[:, ct, :],
                    xTs[ct][jc // 4][:, (jc % 4) * 512:(jc % 4 + 1) * 512],
                    s_sb[:, ct:ct + 1], t_sb[:, ct:ct + 1], OP.mult, OP.add)
            for ct in range(CT):
                pk = pb_ps.tile([128, 512], F32, tag="qkv")
                for k2 in range(2):
                    nc.tensor.matmul(
                        pk[:, :], wkr[:, 2 * k2:2 * k2 + 2, ct * 128:(ct + 1) * 128],
                        hnT[:, 2 * k2:2 * k2 + 2, :],
                        start=(k2 == 0), stop=(k2 == 1), perf_mode=DR)
                # kT eviction split scalar/vector (scalar is the busier
                # engine); the last chunks go all-scalar so the vector queue
                # drains before phase C's hnT->scores handoff.
                if ct % 2 == 0 or jc >= 6:
                    nc.scalar.activation(
                        kT[:, ct, jc * 512:(jc + 1) * 512], pk[:, :],
                        AF.Identity, bias=cv[:, ct, 1:2])
                else:
                    nc.vector.tensor_scalar(
                        kT[:, ct, jc * 512:(jc + 1) * 512], pk[:, :],
                        cv[:, ct, 1:2], None, OP.add)
            if jc < QC:   # rows [0, HALF) are the query rows
                for ct in range(CT):
                    pq = pb_ps.tile([128, 512], F32, tag="qkv")
                    for k2 in range(2):
                        nc.tensor.matmul(
                            pq[:, :], wqr[:, 2 * k2:2 * k2 + 2, ct * 128:(ct + 1) * 128],
                            hnT[:, 2 * k2:2 * k2 + 2, :],
                            start=(k2 == 0), stop=(k2 == 1), perf_mode=DR)
                    nc.scalar.activation(
                        qT[:, ct, jc * 512:(jc + 1) * 512], pq[:, :],
                        AF.Identity, bias=cv[:, ct, 0:1])
            for jp in range(4):
                pv = pb_ps.tile([128, 512], F32, tag="qkv")
                for k2 in range(2):
                    nc.tensor.matmul(
                        pv[:, :], hnT[:, 2 * k2:2 * k2 + 2, jp * 128:(jp + 1) * 128],
                        wvr[:, 2 * k2:2 * k2 + 2, :],
                        start=(k2 == 0), stop=(k2 == 1), perf_mode=DR)
                # bv is folded into bp on the host (softmax rows sum to 1 so
                # attn(v + bv) = attn(v) + bv exactly); eviction is a pure
                # cast, split vector/scalar to balance engine load
                if jp % 2 == 0:
                    nc.vector.tensor_copy(vS[:, jc * 4 + jp, :], pv[:, :])
                else:
                    nc.scalar.activation(
                        vS[:, jc * 4 + jp, :], pv[:, :], AF.Copy)

        with tc.tile_pool(name="pb_sb", bufs=3) as pb, \
             tc.tile_pool(name="pb_ps", bufs=6, space="PSUM") as pb_ps:
            for jc in range(JC):
                qkv_chunk(pb, pb_ps, jc)

        wrnd.release()    # free wq/wk/wv fp8 copies (LIFO with wstage)
        wstage.release()

        # ---------------- phase C: attention + projection + residual --------
        with tc.tile_pool(name="pc_sb", bufs=4) as pcs, \
             tc.tile_pool(name="pc_res", bufs=1) as pcr, \
             tc.tile_pool(name="pc_o", bufs=2) as pco, \
             tc.tile_pool(name="ps_o", bufs=1, space="PSUM") as ps_o, \
             tc.tile_pool(name="ps_s", bufs=2, space="PSUM") as ps_s, \
             tc.tile_pool(name="ps_r", bufs=1, space="PSUM") as ps_r, \
             tc.tile_pool(name="ps_y", bufs=1, space="PSUM") as ps_y:
            NP = JT // 2
            for ib in range(IB):
                po = ps_o.tile([128, CT, 512], F32)
                pr = ps_r.tile([1, 512], F32)
                # prefetch residual rows + bias for this i-block (one DMA)
                xrb = pcr.tile([128, 4, C], F32, tag="xrb")
                nc.sync.dma_start(
                    out=xrb,
                    in_=xq[ib * 512:(ib + 1) * 512, :].rearrange(
                        "(t p) c -> p t c", p=128))
                bpxs = []
                for ip in range(4):
                    bpx = pcr.tile([128, C], F32, tag=f"bpx{ip}")
                    nc.gpsimd.tensor_tensor(
                        bpx[:, :], xrb[:, ip, :], bp_bc[:, :], OP.add)
                    bpxs.append(bpx)

                # software-pipelined j-loop: emit exps(n) BEFORE pv(n-1) and
                # scores(n+1) so the exp's program-order semaphore threshold
                # does not include the PV matmuls (which stalled the PE by
                # ~0.4us per iteration otherwise).
                def scores(n):
                    pair = []
                    for par in range(2):
                        j = 2 * n + par
                        pss = ps_s.tile([128, 512], F32, tag="scores")
                        for k2 in range(2):
                            nc.tensor.matmul(
                                pss[:, :],
                                kT[:, 2 * k2:2 * k2 + 2, j * 128:(j + 1) * 128],
                                qT[:, 2 * k2:2 * k2 + 2, ib * 512:(ib + 1) * 512],
                                start=(k2 == 0), stop=(k2 == 1), perf_mode=DR)
                        pair.append(pss)
                    return pair

                def exps(n, pair):
                    et = pcs.tile([128, 2, 512], F8, tag="exp")
                    for par in range(2):
                        nc.scalar.activation(et[:, par, :], pair[par], AF.Exp,
                                             bias=expb[:, :], scale=SM8)
                    return et

                def pv(n, et):
                    for ct in range(CT):
                        nc.tensor.matmul(
                            po[:, ct, :],
                            vS[:, 2 * n:2 * n + 2, ct * 128:(ct + 1) * 128],
                            et[:, :, :], start=(n == 0), stop=(n == NP - 1),
                            perf_mode=DR)
                    # row-sums of exp: 2.0^T @ etT -> [1, 512] (i on free dim)
                    nc.tensor.matmul(
                        pr[:, :], ones2[:, :, 0:1], et[:, :, :],
                        start=(n == 0), stop=(n == NP - 1), perf_mode=DR)

                pair = scores(0)
                prev_et = None
                for n in range(NP):
                    et = exps(n, pair)
                    if n > 0:
                        pv(n - 1, prev_et)
                    if n + 1 < NP:
                        pair = scores(n + 1)
                    prev_et = et
                pv(NP - 1, prev_et)
                # move the row-sums into per-partition layout [128, 4] via a
                # DRAM bounce (off-engine), then one cheap elementwise divide
                srow = pcs.tile([1, 512], F32, tag="srow")
                nc.scalar.activation(srow[:, :], pr[:, :], AF.Copy)
                nc.gpsimd.dma_start(out=sumscr[ib:ib + 1, :], in_=srow[:, :])
                st4 = pcr.tile([128, IB], F32, tag="st4")
                nc.gpsimd.dma_start(
                    out=st4[:, :],
                    in_=sumscr[ib, :].rearrange("(b p) -> p b", p=128))
                rt = pcr.tile([128, IB], F32, tag="rt")
                nc.vector.reciprocal(rt[:, :], st4[:, :])
                # unnormalized outT eviction (scaled into fp8 range), split
                # scalar/vector so neither engine gates the projection
                ot = pco.tile([128, CT, 512], F8, tag="outT")
                for ct in range(CT):
                    if ct % 2 == 0:
                        nc.scalar.activation(ot[:, ct, :], po[:, ct, :],
                                             AF.Copy, scale=OTS)
                    else:
                        nc.vector.tensor_scalar(ot[:, ct, :], po[:, ct, :],
                                                OTS, None, OP.mult)
                # evict py to SBUF immediately (no rt dependency) so the four
                # projection groups stream through the single PSUM bank
                # without waiting on the row-sum bounce; normalize afterwards
                ycps = []
                for ip in range(4):
                    py = ps_y.tile([128, 512], F32, tag="proj")
                    for c2 in range(2):
                        nc.tensor.matmul(
                            py[:, :], ot[:, 2 * c2:2 * c2 + 2, ip * 128:(ip + 1) * 128],
                            wpr[:, 2 * c2:2 * c2 + 2, :],
                            start=(c2 == 0), stop=(c2 == 1), perf_mode=DR)
                    ycp = pcs.tile([128, C], F32, tag=f"ycp{ip}")
                    if ip % 2 == 0:
                        nc.vector.tensor_copy(ycp[:, :], py[:, :])
                    else:
                        nc.scalar.activation(ycp[:, :], py[:, :], AF.Copy)
                    ycps.append(ycp)
                for ip in range(4):
                    y2 = pcs.tile([128, C], F32, tag="y2")
                    nc.vector.scalar_tensor_tensor(
                        y2[:, :], ycps[ip][:, :], rt[:, ip:ip + 1], bpxs[ip][:, :],
                        OP.mult, OP.add)
                    nc.sync.dma_start(out=yt[ib * 4 + ip, :, :], in_=y2[:, :])

        cst.release()
        store.release()

    if split_waits:
        _split_multi_waits(nc)
    return nc


_PROGRAM = None


def _get_program():
    global _PROGRAM
    if _PROGRAM is None:
        _PROGRAM = build_program()
    return _PROGRAM


def make_in_maps(x, gamma, beta, wq, bq, wk, bk, wv, bv, wp, bp):
    import ml_dtypes
    f32 = lambda a: np.ascontiguousarray(a, dtype=np.float32)
    bf16 = lambda a: np.ascontiguousarray(np.asarray(a, dtype=np.float32).astype(ml_dtypes.bfloat16))
    xr = f32(x).reshape(B, HW, C)
    gsel = np.zeros((GROUPS, C), dtype=np.float32)
    for g in range(GROUPS):
        gsel[g, g * GSIZE:(g + 1) * GSIZE] = 1.0
    gsel2 = np.zeros((128, CT, GROUPS), dtype=np.float32)
    for p in range(128):
        for ct in range(CT):
            gsel2[p, ct, (ct * 128 + p) // GSIZE] = 1.0 / GSIZE
    # packed per-channel constants: cvec[p, ct, :] = (16bq, 16bk, gamma, beta)
    cvec = np.stack([f32(bq) * WSC, f32(bk) * WSC, f32(gamma), f32(beta)],
                    axis=1).reshape(CT, 128, 4).transpose(1, 0, 2)
    common = {
        "wq": bf16(f32(wq) * WSC), "wk": bf16(f32(wk) * WSC),
        "wv": bf16(f32(wv) * WSC), "wp": bf16(f32(wp) * WSC),
        "cvec": np.ascontiguousarray(cvec),
        # bv rides through attention (softmax rows sum to 1): fold into bp
        "bp": f32(bp) + f32(bv) @ f32(wp),
        "gsel": gsel, "gsel2": gsel2,
    }
    in_maps = []
    for c in range(N_CORES):
        b, h = c // 2, c % 2
        m = dict(common)
        if h == 0:
            xp = xr[b]
        else:
            xp = np.concatenate([xr[b, HALF:], xr[b, :HALF]], axis=0)
        # pre-transpose to channel-major, slab-tiled [2, CT, 128, 2048] fp8
        # so each (half, ct) slab DMA is one contiguous 256KB read
        m["xT"] = np.ascontiguousarray(
            xp.T.astype(ml_dtypes.float8_e4m3).reshape(CT, 128, 2, 2048)
            .transpose(2, 0, 1, 3))
        m["xq"] = np.ascontiguousarray(xr[b, h * HALF:(h + 1) * HALF])
        in_maps.append(m)
    return in_maps


def kernel(x, gamma, beta, wq, bq, wk, bk, wv, bv, wp, bp, _trace=False):
    nc = _get_program()
    in_maps = make_in_maps(x, gamma, beta, wq, bq, wk, bk, wv, bv, wp, bp)
    res = run_bass_kernel_spmd(nc, in_maps, list(range(N_CORES)), trace=_trace)
    out = np.empty((B, HW, C), dtype=np.float32)
    for c in range(N_CORES):
        b, h = c // 2, c % 2
        out[b, h * HALF:(h + 1) * HALF] = res.results[c]["y"]
    if _trace:
        kernel._last_result = res
    return out.reshape(B, H, W, C)
